# revision 11
# baseline (speedup 1.0000x reference)
"""Depth-weighted average pooling (3x3, stride 2) on 8 Trainium2 NeuronCores.

out[n,c,ho,wo] = sum_ij x[n,c,2ho+i,2wo+j] * w_ij / sum_ij w_ij
  w_ij = exp(-|d[n,2ho+1,2wo+1] - d[n,2ho+i,2wo+j]|)

Sharding: batch N=8, one image per core (data parallel, no halo).

Per-core layout ("rows mod 4" mapping): SBUF partition p holds input rows
4p..4p+3 (tile dim t) plus a re-read of row 4p+4 (T0'), so partition p
computes output rows 2p ("even sub", input rows 4p..4p+2) and 2p+1
("odd sub", rows 4p+2..4p+4).  All engine access patterns start at
partition 0 (hardware requires start partition in {0,32,64,96}).
Weights are computed in the same [p, sub, wo] layout, so they are direct
per-partition operands of the vector engine — no cross-partition
broadcast is ever needed.  Channels live in the free dimension.
"""

import os
import sys
import functools

import numpy as np

for _p in ("/opt/trn_rl_repo", "/opt/trn_rl_repo/concourse"):
    if os.path.isdir(_p) and _p not in sys.path:
        sys.path.insert(0, _p)

KH = KW = 3
SH = SW = 2
N_CORES = 8
C, H, W = 64, 512, 512

# taps in order; (1,1) is the center
TAPS = [(i, j) for i in range(3) for j in range(3)]
NC_TAPS = [t for t in TAPS if t != (1, 1)]


def _main_bf16(nc, tc, xp, pp, ap_, x, o, wmb, C, G, W, Ho, Wo, Wp, PE, PO):
    """bf16 main channel loop.

    Per group: SWDGE cast-DMA loads x rows as contiguous bf16; ScalarE
    de-interleaves each row into three 4B-aligned tap arrays
    [j0 | j1 | j2] (each Wp wide) so every vector op runs in the packed
    2x bf16 mode; the odd-sub i=2 operand is built by a partition-shift
    SBUF->SBUF DMA of the expanded tile.  Accumulation in bf16; the
    store casts back to fp32 in the DMA.
    """
    from concourse import mybir

    bf16 = mybir.dt.bfloat16

    for g in range(C // G):
        c0 = g * G
        XC = xp.tile([PE, 4, G, W], bf16, tag="XC")
        nc.gpsimd.dma_start(
            out=XC[:],
            in_=x[c0 : c0 + G, 0 : 4 * PE, :].rearrange("c (p t) w -> p t c w", t=4),
        )
        # expand rows t=0..3: XB[p,t,c] = [x[..,0::2] | x[..,1::2] | x[..,2::2] pad]
        # t=4 = expanded row 4p+4 (next partition's t=0) via DMA partition shift
        XB = xp.tile([PE, 5, G, 3 * Wp], bf16, tag="XB")
        nc.scalar.copy(XB[0:PE, 0:4, :, 0:Wp], XC[0:PE, :, :, 0 : 2 * Wp : 2])
        nc.scalar.copy(XB[0:PE, 0:4, :, Wp : 2 * Wp], XC[0:PE, :, :, 1 : 2 * Wp : 2])
        nc.scalar.copy(
            XB[0:PE, 0:4, :, 2 * Wp : 2 * Wp + Wo],
            XC[0:PE, :, :, 2 : 2 + 2 * Wo : 2],
        )
        # defined (finite) pad column for j2
        nc.scalar.copy(XB[0:PE, 0:4, :, 3 * Wp - 1 : 3 * Wp], XC[0:PE, :, :, 0:1])
        nc.sync.dma_start(out=XB[0:PO, 4], in_=XB[1 : PO + 1, 0])

        def xv_m(i, j):  # [PE, G, 2, Wp]; sub stride = 2 slots in t
            return XB[0:PE, i : i + 3 : 2, :, j * Wp : (j + 1) * Wp].transpose(
                [0, 2, 1, 3]
            )

        def wv_m(wb):
            return wb[0:PE].unsqueeze(1).broadcast_to([PE, G, 2, Wp])

        acc = ap_.tile([PE, G, 2, Wp], bf16, tag="acc")
        nc.vector.tensor_mul(acc[:], xv_m(1, 1), wv_m(wmb[(1, 1)]))
        for (i, j) in NC_TAPS:
            p = pp.tile([PE, G, 2, Wp], bf16, tag="pm")
            nc.vector.tensor_mul(p[:], xv_m(i, j), wv_m(wmb[(i, j)]))
            nc.vector.tensor_add(acc[:], acc[:], p[:])

        # cast back to fp32 in the store DMA (SWDGE)
        nc.gpsimd.dma_start(
            out=o[c0 : c0 + G, 0 : Ho : 2, :].transpose([1, 0, 2]),
            in_=acc[0:PE, :, 0, 0:Wo],
        )
        nc.gpsimd.dma_start(
            out=o[c0 : c0 + G, 1 : Ho : 2, :].transpose([1, 0, 2]),
            in_=acc[0:PO, :, 1, 0:Wo],
        )


def build_kernel(C=C, H=H, W=W, G=4, repeat=1, variant="fp32"):
    """Single-core Bass program: x[C,H,W], d[H,W] -> o[C,Ho,Wo]."""
    from contextlib import ExitStack

    import concourse.bacc as bacc
    from concourse.tile import TileContext
    from concourse import mybir

    f32 = mybir.dt.float32
    bf16 = mybir.dt.bfloat16
    AluOp = mybir.AluOpType
    Act = mybir.ActivationFunctionType
    Wp = W // 2  # padded output-width for bf16 tiles (= Wo+1, even)

    Ho = (H - KH) // SH + 1
    Wo = (W - KW) // SW + 1
    assert C % G == 0 and H % 4 == 0
    PE = (Ho + 1) // 2  # partitions carrying an even-sub output row
    PO = Ho // 2  # partitions carrying an odd-sub output row
    assert PE <= 128 and Ho == 2 * PE - 1

    nc = bacc.Bacc(
        "TRN2",
        target_bir_lowering=False,
        debug=False,
        enable_asserts=False,
        num_devices=1,
    )
    x = nc.dram_tensor("x", [C, H, W], f32, kind="ExternalInput").ap()
    d = nc.dram_tensor("d", [H, W], f32, kind="ExternalInput").ap()
    o = nc.dram_tensor("o", [C, Ho, Wo], f32, kind="ExternalOutput").ap()

    with TileContext(nc) as tc, ExitStack() as ctx:
        xp = ctx.enter_context(tc.tile_pool(name="xp", bufs=2))
        dp = ctx.enter_context(tc.tile_pool(name="dp", bufs=1))
        wp = ctx.enter_context(tc.tile_pool(name="wp", bufs=1))
        tp = ctx.enter_context(tc.tile_pool(name="tp", bufs=1))
        pp = ctx.enter_context(tc.tile_pool(name="pp", bufs=1))
        ap_ = ctx.enter_context(tc.tile_pool(name="ap", bufs=2))

        for _rep in range(repeat):
            # ---- depth tiles: DT[p, t, w] = d[4p+t, w]; D4[p, w] = d[4p+4, w]
            DT = dp.tile([PE, 4, W], f32, tag="DT")
            nc.sync.dma_start(
                out=DT[:], in_=d[0 : 4 * PE, :].rearrange("(p t) w -> p t w", t=4)
            )
            D4 = dp.tile([PO, W], f32, tag="D4")
            nc.sync.dma_start(out=D4[:], in_=d[4 : 4 * PO + 1 : 4, :])

            # window-center depth, both subs: dc[p, s, wo] = d[4p+2s+1, 2wo+1]
            dcm = DT[0:PE, 1:4:2, 1 : 1 + 2 * Wo : 2]  # [PE, 2, Wo]
            dce = DT[0:PE, 1, 1 : 1 + 2 * Wo : 2]  # [PE, Wo]
            dco = DT[0:PO, 3, 1 : 1 + 2 * Wo : 2]  # [PO, Wo]

            # ---- 8 non-center weight maps wm[p, s, wo] (normalized later)
            wm = {}
            for (i, j) in NC_TAPS:
                wt = wp.tile([PE, 2, Wo], f32, tag=f"w{i}{j}")
                if i < 2:
                    dv = DT[0:PE, i : i + 3 : 2, j : j + 2 * Wo : 2]
                    df = tp.tile([PE, 2, Wo], f32, tag="df")
                    nc.vector.tensor_sub(df[:], dcm, dv)
                    ab = tp.tile([PE, 2, Wo], f32, tag="ab")
                    nc.vector.scalar_tensor_tensor(
                        ab[:], df[:], -1.0, df[:], AluOp.mult, AluOp.max
                    )
                    nc.scalar.activation(wt[:], ab[:], Act.Exp, scale=-1.0)
                else:
                    # even sub from DT row t=2; odd sub from D4; pad rows -> 0
                    nc.vector.memzero(wt[:])
                    dfe = tp.tile([PE, Wo], f32, tag="dfe")
                    nc.vector.tensor_sub(dfe[:], dce, DT[0:PE, 2, j : j + 2 * Wo : 2])
                    abe = tp.tile([PE, Wo], f32, tag="abe")
                    nc.vector.scalar_tensor_tensor(
                        abe[:], dfe[:], -1.0, dfe[:], AluOp.mult, AluOp.max
                    )
                    nc.scalar.activation(wt[0:PE, 0, :], abe[:], Act.Exp, scale=-1.0)
                    dfo = tp.tile([PO, Wo], f32, tag="dfo")
                    nc.vector.tensor_sub(dfo[:], dco, D4[0:PO, j : j + 2 * Wo : 2])
                    abo = tp.tile([PO, Wo], f32, tag="abo")
                    nc.vector.scalar_tensor_tensor(
                        abo[:], dfo[:], -1.0, dfo[:], AluOp.mult, AluOp.max
                    )
                    nc.scalar.activation(wt[0:PO, 1, :], abo[:], Act.Exp, scale=-1.0)
                wm[(i, j)] = wt

            # ---- den = 1 + sum of the 8 maps; rden = 1/den
            ks = list(wm)
            s01 = tp.tile([PE, 2, Wo], f32, tag="s01")
            nc.vector.tensor_add(s01[:], wm[ks[0]][:], wm[ks[1]][:])
            s23 = tp.tile([PE, 2, Wo], f32, tag="s23")
            nc.vector.tensor_add(s23[:], wm[ks[2]][:], wm[ks[3]][:])
            s45 = tp.tile([PE, 2, Wo], f32, tag="s45")
            nc.vector.tensor_add(s45[:], wm[ks[4]][:], wm[ks[5]][:])
            s67 = tp.tile([PE, 2, Wo], f32, tag="s67")
            nc.vector.tensor_add(s67[:], wm[ks[6]][:], wm[ks[7]][:])
            nc.vector.tensor_add(s01[:], s01[:], s23[:])
            nc.vector.tensor_add(s45[:], s45[:], s67[:])
            nc.vector.tensor_add(s01[:], s01[:], s45[:])
            den = tp.tile([PE, 2, Wo], f32, tag="den")
            nc.vector.tensor_scalar_add(den[:], s01[:], 1.0)
            rden = wp.tile([PE, 2, Wo], f32, tag="rden")
            nc.vector.reciprocal(rden[:], den[:])
            # normalize in place; center weight becomes rden itself
            for wt in wm.values():
                nc.vector.tensor_mul(wt[:], wt[:], rden[:])

            if variant == "bf16":
                # convert the 9 normalized maps to padded bf16 tiles
                wmb = {}
                for (i, j) in NC_TAPS + [(1, 1)]:
                    src = rden if (i, j) == (1, 1) else wm[(i, j)]
                    wb = wp.tile([PE, 2, Wp], bf16, tag=f"wb{i}{j}")
                    nc.vector.memzero(wb[:])
                    nc.scalar.copy(wb[0:PE, :, 0:Wo], src[:])
                    wmb[(i, j)] = wb
                _main_bf16(nc, tc, xp, pp, ap_, x, o, wmb, C, G, W, Ho, Wo, Wp, PE, PO)
                continue

            # ---- main channel loop
            for g in range(C // G):
                c0 = g * G
                XT = xp.tile([PE, 4, G, W], f32, tag="XT")
                nc.sync.dma_start(
                    out=XT[:],
                    in_=x[c0 : c0 + G, 0 : 4 * PE, :].rearrange(
                        "c (p t) w -> p t c w", t=4
                    ),
                )
                X4 = xp.tile([PO, G, W], f32, tag="X4")
                nc.sync.dma_start(
                    out=X4[:],
                    in_=x[c0 : c0 + G, 4 : 4 * PO + 1 : 4, :].transpose([1, 0, 2]),
                )

                # x tap views, merged over subs where both live in XT
                def xv_m(i, j):  # [PE, G, 2, Wo]; sub stride = 2 rows in t
                    return XT[0:PE, i : i + 3 : 2, :, j : j + 2 * Wo : 2].transpose(
                        [0, 2, 1, 3]
                    )

                def xv_e(j):  # even sub of tap i=2: XT row t=2
                    return XT[0:PE, 2, :, j : j + 2 * Wo : 2]

                def xv_o(j):  # odd sub of tap i=2: X4
                    return X4[0:PO, :, j : j + 2 * Wo : 2]

                def wv_m(wt):  # [PE, G, 2, Wo] broadcast over channels
                    return wt[0:PE].unsqueeze(1).broadcast_to([PE, G, 2, Wo])

                acc = ap_.tile([PE, G, 2, Wo], f32, tag="acc")
                # center tap: acc = x_center * rden
                nc.vector.tensor_mul(acc[:], xv_m(1, 1), wv_m(rden))
                for (i, j) in NC_TAPS:
                    wt = wm[(i, j)]
                    if i < 2:
                        p = pp.tile([PE, G, 2, Wo], f32, tag="pm")
                        nc.vector.tensor_mul(p[:], xv_m(i, j), wv_m(wt))
                        nc.vector.tensor_add(acc[:], acc[:], p[:])
                    else:
                        pe = pp.tile([PE, G, Wo], f32, tag="pe")
                        we = wt[0:PE, 0].unsqueeze(1).broadcast_to([PE, G, Wo])
                        nc.vector.tensor_mul(pe[:], xv_e(j), we)
                        nc.vector.tensor_add(
                            acc[0:PE, :, 0, :], acc[0:PE, :, 0, :], pe[:]
                        )
                        po = pp.tile([PO, G, Wo], f32, tag="po")
                        wo_ = wt[0:PO, 1].unsqueeze(1).broadcast_to([PO, G, Wo])
                        nc.vector.tensor_mul(po[:], xv_o(j), wo_)
                        nc.vector.tensor_add(
                            acc[0:PO, :, 1, :], acc[0:PO, :, 1, :], po[:]
                        )

                # ---- store: even rows ho=2p, odd rows ho=2p+1
                nc.sync.dma_start(
                    out=o[c0 : c0 + G, 0 : Ho : 2, :].transpose([1, 0, 2]),
                    in_=acc[0:PE, :, 0, :],
                )
                nc.sync.dma_start(
                    out=o[c0 : c0 + G, 1 : Ho : 2, :].transpose([1, 0, 2]),
                    in_=acc[0:PO, :, 1, :],
                )

    nc.compile()
    return nc


@functools.lru_cache(maxsize=4)
def _compiled(key):
    C_, H_, W_, G, repeat, variant = key
    return build_kernel(C=C_, H=H_, W=W_, G=G, repeat=repeat, variant=variant)


def kernel(input, depth):
    """Full-io entry: input [8,64,512,512] f32, depth [8,1,512,512] f32."""
    from concourse import bass_utils

    input = np.ascontiguousarray(np.asarray(input), dtype=np.float32)
    depth = np.ascontiguousarray(np.asarray(depth), dtype=np.float32)
    N = input.shape[0]
    assert N == N_CORES and input.shape[1:] == (C, H, W)

    nc = _compiled((C, H, W, 4, 1, "fp32"))
    in_maps = [{"x": input[n], "d": depth[n, 0]} for n in range(N)]
    res = bass_utils.run_bass_kernel_spmd(nc, in_maps, core_ids=list(range(N)))
    out = np.stack([r["o"] for r in res.results], axis=0)
    return out


if __name__ == "__main__":
    nc = build_kernel()
    print("built ok")
